# revision 15
# baseline (speedup 1.0000x reference)
"""Trainium2 Bass kernel for nn_Attention (B=2, S=2048, D=2048, H=16, causal).

v4: overlaps attention for strips 0-2 INTO phase 1's PE-dense window.
Phase 1 shrinks to 3 PSUM banks (2-head q/k subpasses, 1-chunk v subpasses);
a minimal attention set (pl [128,1024] x1, po x2, pd x1) coexists. Attention
units are emitted in two stages (logits+exp | mask+PV+dn) a block apart so
the PV never waits on an in-flight exp at the head of the PE queue. Strip-3
attention + the whole output projection run in a tail scope with full-width
pools, with ph3(0..2) tiles as interleave fill.

Sharding: batch x head-group. Core c owns batch b=c//4 and heads [4g, 4g+4).
Host sums 4 bf16 partials per batch (+ bo).
"""

import math
import os
import sys

sys.path.insert(0, "/opt/trn_rl_repo")
os.environ.setdefault("BASS_NEVER_TRACE", "1")

import numpy as np

import concourse.bass as bass
import concourse.tile as tile
from concourse import bacc, mybir
from concourse.bass_utils import run_bass_kernel_spmd

F32 = mybir.dt.float32
BF16 = mybir.dt.bfloat16

P = 128
B, S, D, H = 2, 2048, 2048, 16
HD = 128                  # head dim
NH = 4                    # heads per core
TOK = S                   # tokens per core (one batch)
QS = 512                  # strip width
NSTRIP = TOK // QS        # 4
CC = D // P               # 16 contraction chunks
WSCALE = 32.0             # host premultiplies Wqkv so fp8 splits stay normal
SCALE = 1.0 / math.sqrt(HD) / (WSCALE * WSCALE)  # exp absorbs q,k scaling

_NC_CACHE = {}


F8 = mybir.dt.float8e4
CCP = CC // 2             # 8 k-slab pairs for fp8 DoubleRow


def _build_nc(dump=False, reps=1):
    nc = bacc.Bacc("TRN2", target_bir_lowering=False, debug=False, num_devices=8)
    # x / Wqkv arrive as fp8 hi+lo splits packed for DoubleRow:
    #   xq  [P, strip, ccpair, (x8 s0, x8 s1, dx8 s0, dx8 s1), QS]
    #   w*  [P, ccpair, (w8 s0, w8 s1, dw8 s0, dw8 s1), NH*HD]
    # qkv = x8@w8 + dx8@w8 + x8@dw8 (+dx8@dw8 dropped) == bf16-level accuracy
    # at fp8-DoubleRow speed.
    xq = nc.dram_tensor("xq", [P, NSTRIP * CCP * 4 * QS], F8,
                        kind="ExternalInput").ap()
    wq = nc.dram_tensor("wq", [P, CCP * 4 * NH * HD], F8,
                        kind="ExternalInput").ap()
    wk = nc.dram_tensor("wk", [P, CCP * 4 * NH * HD], F8,
                        kind="ExternalInput").ap()
    wv = nc.dram_tensor("wv", [P, CCP * 4 * NH * HD], F8,
                        kind="ExternalInput").ap()
    wo = nc.dram_tensor("wo", [NH * HD, D], BF16, kind="ExternalInput").ap()
    out = nc.dram_tensor("out", [TOK, D], BF16, kind="ExternalOutput").ap()

    import contextlib
    with tile.TileContext(nc) as tc:
        with (tc.For_i(0, reps, 1) if reps > 1 else contextlib.nullcontext()), \
             tc.tile_pool(name="resid", bufs=1) as resid, \
             tc.tile_pool(name="const", bufs=1) as const, \
             tc.tile_pool(name="stp", bufs=6) as stp, \
             tc.tile_pool(name="dnp", bufs=3) as dnp, \
             tc.tile_pool(name="evp", bufs=3) as evp, \
             tc.tile_pool(name="outp", bufs=6) as outp:
            qTs = [resid.tile([P, NH * QS], BF16, name=f"qT{_s}")
                   for _s in range(NSTRIP)]
            kTs = [resid.tile([P, NH * QS], BF16, name=f"kT{_s}")
                   for _s in range(NSTRIP)]
            vNs = [resid.tile([P, (QS // P) * NH * HD], BF16, name=f"vN{_s}")
                   for _s in range(NSTRIP)]
            attnTs = {(_h, _qi): resid.tile([P, QS], BF16,
                                            name=f"at{_h}_{_qi}")
                      for _h in range(NH) for _qi in range(NSTRIP)}
            wo_sb = resid.tile([P, NH * D], BF16, name="wo_sb")
            ones_f32 = const.tile([P, 1], F32)
            nc.gpsimd.memset(ones_f32[:], 1.0)
            ones = const.tile([P, 1], BF16)
            nc.vector.tensor_copy(ones[:], ones_f32[:])
            masks_f = const.tile([P, QS], F32)
            nc.gpsimd.memset(masks_f[:], 1.0)
            nc.gpsimd.affine_select(
                out=masks_f[:], in_=masks_f[:],
                compare_op=mybir.AluOpType.is_ge, fill=0.0,
                base=0, channel_multiplier=-1, pattern=[[1, QS]],
            )
            masks = const.tile([P, QS], BF16)
            nc.vector.tensor_copy(masks[:], masks_f[:])

            # --- attention stage machinery (shared by overlap and tail) ---
            # state[h] = dict(po, dn, dnb, pl, st, unit)
            def make_attn(qi, psl_pool, acc_pool, psd_pool):
                state = {}

                def open_h(h):
                    def f():
                        state[h] = {
                            "po": acc_pool.tile([P, QS], F32, tag="acc",
                                                name="po"),
                            "dn": dnp.tile([P, QS], BF16, tag="dn", name="dn"),
                            "dnb": dnp.tile([P, QS], BF16, tag="dnb",
                                            name="dnb") if qi > 0 else None,
                        }
                    return f

                def s1(h, kind, idx):
                    # logits + exp for one unit; stashes pl/st in state
                    def f():
                        st_ = state[h]
                        pl2 = psl_pool.tile([P, 2 * QS], F32, tag="pl",
                                            name="pl")
                        if kind == "pair":
                            for hf in range(2):
                                j = 2 * idx + hf
                                js, jc = j // 4, j % 4
                                nc.tensor.matmul(
                                    pl2[:, hf * QS:(hf + 1) * QS],
                                    kTs[js][:, h * QS + jc * P:
                                            h * QS + (jc + 1) * P],
                                    qTs[qi][:, h * QS:(h + 1) * QS],
                                    start=True, stop=True)
                            st2 = stp.tile([P, 2 * QS], BF16, tag="st",
                                           name="st")
                            nc.scalar.activation(
                                st2[:], pl2[:],
                                mybir.ActivationFunctionType.Exp, scale=SCALE)
                        else:  # diag
                            pi = idx
                            j = 4 * qi + pi
                            js, jc = j // 4, j % 4
                            c0 = pi * P
                            w = QS - c0
                            nc.tensor.matmul(
                                pl2[:, :w],
                                kTs[js][:, h * QS + jc * P:
                                        h * QS + (jc + 1) * P],
                                qTs[qi][:, h * QS + c0:(h + 1) * QS],
                                start=True, stop=True)
                            st2 = stp.tile([P, 2 * QS], BF16, tag="st",
                                           name="st")
                            nc.scalar.activation(
                                st2[:, :w], pl2[:, :w],
                                mybir.ActivationFunctionType.Exp, scale=SCALE)
                        st_["st"] = st2
                    return f

                def s2(h, kind, idx):
                    # mask + PV + dn for the unit s1 stashed
                    def f():
                        st_ = state[h]
                        st2 = st_["st"]
                        po, dn, dnb = st_["po"], st_["dn"], st_["dnb"]
                        nj = 4 * qi + 4
                        if kind == "pair":
                            p = idx
                            for hf in range(2):
                                j = 2 * p + hf
                                js, jc = j // 4, j % 4
                                nc.tensor.matmul(
                                    po[:],
                                    vNs[js][:, jc * QS + h * HD:
                                            jc * QS + (h + 1) * HD],
                                    st2[:, hf * QS:(hf + 1) * QS],
                                    start=(j == 0), stop=False)
                            if p == 0:
                                nc.vector.tensor_copy(dn[:], st2[:, :QS])
                                nc.vector.tensor_copy(dnb[:], st2[:, QS:])
                            else:
                                nc.vector.tensor_add(
                                    dn[:], dn[:], st2[:, :QS])
                                nc.vector.tensor_add(
                                    dnb[:], dnb[:], st2[:, QS:])
                        else:
                            pi = idx
                            j = 4 * qi + pi
                            js, jc = j // 4, j % 4
                            c0 = pi * P
                            w = QS - c0
                            nc.vector.tensor_mul(
                                st2[:, :w], st2[:, :w], masks[:, :w])
                            nc.tensor.matmul(
                                po[:, c0:],
                                vNs[js][:, jc * QS + h * HD:
                                        jc * QS + (h + 1) * HD],
                                st2[:, :w],
                                start=(j == 0), stop=(pi == 3))
                            if qi == 0 and pi == 0:
                                nc.vector.tensor_copy(dn[:], st2[:, :QS])
                            elif qi > 0 and pi % 2 == 1:
                                nc.vector.tensor_add(
                                    dnb[:, c0:], dnb[:, c0:], st2[:, :w])
                            else:
                                nc.vector.tensor_add(
                                    dn[:, c0:], dn[:, c0:], st2[:, :w])
                    return f

                def close_h(h):
                    def f():
                        st_ = state[h]
                        po, dn, dnb = st_["po"], st_["dn"], st_["dnb"]
                        if dnb is not None:
                            nc.vector.tensor_add(dn[:], dn[:], dnb[:])
                        pd = psd_pool.tile([1, QS], F32, tag="pd", name="pd")
                        nc.tensor.matmul(pd[:], ones[:], dn[:],
                                         start=True, stop=True)
                        rc = evp.tile([1, QS], F32, tag="rc")
                        nc.vector.reciprocal(rc[:], pd[:])
                        bc = evp.tile([P, QS], F32, tag="bc")
                        nc.gpsimd.partition_broadcast(bc[:], rc[:])
                        nc.vector.tensor_mul(
                            attnTs[(h, qi)][:], po[:], bc[:])
                    return f

                # quanta: [open+s1(u0)], [s2(u0), s1(u1)], ..., [s2(un), close]
                quanta = []
                for h in range(NH):
                    units = [("pair", p) for p in range(2 * qi)] + \
                            [("diag", pi) for pi in range(4)]
                    quanta.append([open_h(h), s1(h, *units[0])])
                    for i in range(1, len(units)):
                        quanta.append([s2(h, *units[i - 1]), s1(h, *units[i])])
                    quanta.append([s2(h, *units[-1]), close_h(h)])
                return quanta

            def ph3_tile(qi, t, n, acc_pool):
                tok0 = qi * QS
                pf = acc_pool.tile([P, QS], F32, tag="acc", name="pf")
                for h in range(NH):
                    at = attnTs[(h, qi)]
                    nc.tensor.matmul(
                        pf[:],
                        at[:, t * P:(t + 1) * P],
                        wo_sb[:, h * D + n * QS: h * D + (n + 1) * QS],
                        start=(h == 0), stop=(h == NH - 1))
                ot = outp.tile([P, QS], BF16, tag="ot", name="ot")
                if n == 3:
                    nc.scalar.copy(ot[:], pf[:])
                else:
                    nc.vector.tensor_copy(ot[:], pf[:])
                oeng = nc.sync if n % 2 == 0 else nc.gpsimd
                oeng.dma_start(
                    out[tok0 + t * P: tok0 + (t + 1) * P,
                        n * QS:(n + 1) * QS], ot[:])

            # ---------- Phase 1 + overlapped attention (strips 0-2) -------
            # v5: QKV as per-head 16-cc chains, each accumulating into ONE
            # PSUM tile consecutively. Measured on hw: matmuls streaming
            # into the same PSUM region run ~2x faster (0.225 ns/row) than
            # target-alternating sequences (0.42 ns/row), so chain-per-head
            # beats the old 2-head-interleaved subpasses.
            with tc.tile_pool(name="wpool", bufs=1) as wpool, \
                 tc.tile_pool(name="xpool", bufs=16) as xpool, \
                 tc.tile_pool(name="psqk", bufs=3, space="PSUM") as psqk, \
                 tc.tile_pool(name="psl_a", bufs=1, space="PSUM") as psl_a, \
                 tc.tile_pool(name="acc_a", bufs=2, space="PSUM") as acc_a, \
                 tc.tile_pool(name="psd_a", bufs=1, space="PSUM") as psd_a:
                wdram = {"wq": wq, "wk": wk, "wv": wv}
                weng = {"wq": nc.scalar, "wk": nc.gpsimd, "wv": nc.gpsimd}
                wsb = {}
                for wn in ("wq", "wk", "wv"):
                    wt = wpool.tile([P, CCP, 4, NH * HD], F8, name=f"{wn}_sb")
                    weng[wn].dma_start(
                        wt[:], wdram[wn].rearrange(
                            "p (c f n) -> p c f n", c=CCP, f=4))
                    wsb[wn] = wt

                pend = []      # quanta of the previous strip's attention
                pend_done = [0]

                def hook(k, ktot):
                    # emit pending attention quanta paced over the strip
                    target = min(len(pend), (k + 1) * len(pend) // ktot)
                    while pend_done[0] < target:
                        for op in pend[pend_done[0]]:
                            op()
                        pend_done[0] += 1

                DR = mybir.MatmulPerfMode.DoubleRow
                for ns in range(NSTRIP):
                    xts = []
                    for cp in range(CCP):
                        xt = xpool.tile([P, 4, QS], F8, tag="xt", name="xt")
                        o = (ns * CCP + cp) * 4 * QS
                        nc.sync.dma_start(
                            xt[:], xq[:, o:o + 4 * QS].rearrange(
                                "p (f n) -> p f n", f=4))
                        xts.append(xt)
                    if ns == 1:
                        nc.sync.dma_start(
                            wo_sb[:].rearrange("p (h n) -> p h n", h=NH),
                            wo.rearrange("(h p) n -> p h n", p=P))
                    hk = [0]
                    KTOT = 12

                    def blk():
                        hook(hk[0], KTOT)
                        hk[0] += 1

                    def qk_chain(wn, tgt, h):
                        # one head's full-K 3-term fp8 chain into one PSUM
                        # tile: x8@w8, dx8@w8, x8@dw8
                        pq = psqk.tile([P, QS], F32, tag="qkv",
                                       name=f"p{wn}{h}")
                        wt = wsb[wn]
                        hs = slice(h * HD, (h + 1) * HD)
                        for tm, (wf, xf) in enumerate(
                                ((0, 0), (0, 2), (2, 0))):
                            for cp in range(CCP):
                                nc.tensor.matmul(
                                    pq[:],
                                    wt[:, cp, wf:wf + 2, hs],
                                    xts[cp][:, xf:xf + 2, :],
                                    start=(tm == 0 and cp == 0),
                                    stop=(tm == 2 and cp == CCP - 1),
                                    perf_mode=DR)
                        nc.vector.tensor_copy(
                            tgt[:, h * QS:(h + 1) * QS], pq[:])
                        blk()

                    def v_chain(t):
                        pv = psqk.tile([P, QS], F32, tag="qkv", name=f"pv{t}")
                        ts = slice(t * P, (t + 1) * P)
                        for tm, (xf, wf) in enumerate(
                                ((0, 0), (2, 0), (0, 2))):
                            for cp in range(CCP):
                                nc.tensor.matmul(
                                    pv[:],
                                    xts[cp][:, xf:xf + 2, ts],
                                    wsb["wv"][:, cp, wf:wf + 2, :],
                                    start=(tm == 0 and cp == 0),
                                    stop=(tm == 2 and cp == CCP - 1),
                                    perf_mode=DR)
                        nc.vector.tensor_copy(
                            vNs[ns][:, t * QS:(t + 1) * QS], pv[:])
                        blk()

                    for _h in range(NH):
                        qk_chain("wq", qTs[ns], _h)
                    for _h in range(NH):
                        qk_chain("wk", kTs[ns], _h)
                    for _t in range(4):
                        v_chain(_t)
                    # drain any leftover quanta, then queue this strip's
                    # attention for the next phase-1 strip (strips 0-2)
                    hook(KTOT - 1, KTOT)
                    if ns < NSTRIP - 1:
                        pend = make_attn(ns, psl_a, acc_a, psd_a)
                        pend_done[0] = 0
                # strip 2's attention has no phase-1 strip left to hide
                # under; emit it here (still inside the overlap pools)
                hook(0, 1)

            # ---------- tail: strip-3 attention + all of phase 3 ----------
            with tc.tile_pool(name="psl", bufs=2, space="PSUM") as psl, \
                 tc.tile_pool(name="acc", bufs=3, space="PSUM") as accp, \
                 tc.tile_pool(name="psdp", bufs=1, space="PSUM") as psdp:
                ph3_pend = [(qi, t, n) for qi in range(NSTRIP - 1)
                            for t in range(QS // P) for n in range(D // QS)]
                quanta = make_attn(NSTRIP - 1, psl, accp, psdp)
                npend = len(ph3_pend)
                done = 0
                reserve = 6
                navail = npend - reserve
                for i, q in enumerate(quanta):
                    for op in q:
                        op()
                    while done < (i + 1) * navail // len(quanta):
                        ph3_tile(*ph3_pend[done], accp)
                        done += 1
                while done < npend:
                    ph3_tile(*ph3_pend[done], accp)
                    done += 1
                for t in range(QS // P):
                    for n in range(D // QS):
                        ph3_tile(NSTRIP - 1, t, n, accp)
    nc.compile()
    return nc


def get_nc(dump=False, reps=1):
    key = ("nc", dump, reps)
    if key not in _NC_CACHE:
        _NC_CACHE[key] = _build_nc(dump, reps)
    return _NC_CACHE[key]


def _to_bf16(a):
    import ml_dtypes
    return np.asarray(a, dtype=ml_dtypes.bfloat16)


def _f8_split(a):
    """fp32 array -> (hi, lo) e4m3 pair with hi+lo ~ a (7-bit mantissa)."""
    import ml_dtypes
    hi = a.astype(ml_dtypes.float8_e4m3)
    lo = (a - hi.astype(np.float32)).astype(ml_dtypes.float8_e4m3)
    return hi, lo


def _prep_in_maps(x, Wqkv):
    in_maps = []
    for c in range(8):
        b, g = c // 4, c % 4
        heads = range(4 * g, 4 * g + 4)
        xT = np.ascontiguousarray(x[b].T)               # [D, TOK] f32
        x8, dx8 = _f8_split(xT)
        # pack [P, strip, ccpair, (x8 s0, x8 s1, dx8 s0, dx8 s1), QS]
        a = x8.reshape(CCP, 2, P, NSTRIP, QS)
        d = dx8.reshape(CCP, 2, P, NSTRIP, QS)
        packed = np.stack([a, d], axis=1)               # (cp, xd, sub, p, s, n)
        m = {"xq": np.ascontiguousarray(
            packed.transpose(3, 4, 0, 1, 2, 5)
            .reshape(P, NSTRIP * CCP * 4 * QS))}
        for name, off in (("wq", 0), ("wk", HD), ("wv", 2 * HD)):
            w = np.concatenate(
                [Wqkv[:, h * 3 * HD + off: h * 3 * HD + off + HD]
                 for h in heads], axis=1)               # [D, 512] f32
            w8, dw8 = _f8_split(w * WSCALE)
            aw = w8.reshape(CCP, 2, P, NH * HD)
            dw = dw8.reshape(CCP, 2, P, NH * HD)
            pk = np.stack([aw, dw], axis=1)             # (cp, wd, sub, p, n)
            m[name] = np.ascontiguousarray(
                pk.transpose(3, 0, 1, 2, 4)
                .reshape(P, CCP * 4 * NH * HD))
        in_maps.append(m)
    return in_maps


def _prep_wo(in_maps, Wo):
    for c in range(8):
        g = c % 4
        in_maps[c]["wo"] = _to_bf16(
            np.ascontiguousarray(Wo[g * NH * HD:(g + 1) * NH * HD, :]))


def kernel(x, Wqkv, bqkv, Wo, bo, _trace=False, _dump=False):
    x = np.asarray(x, dtype=np.float32)
    Wqkv = np.asarray(Wqkv, dtype=np.float32)
    bqkv = np.asarray(bqkv, dtype=np.float32)
    Wo = np.asarray(Wo, dtype=np.float32)
    bo = np.asarray(bo, dtype=np.float32)
    assert not np.any(bqkv), "kernel assumes bqkv == 0"

    in_maps = _prep_in_maps(x, Wqkv)
    _prep_wo(in_maps, Wo)

    nc = get_nc(_dump)
    res = run_bass_kernel_spmd(nc, in_maps, list(range(8)), trace=_trace)
    outs = np.zeros((B, S, D), dtype=np.float32)
    for c in range(8):
        outs[c // 4] += res.results[c]["out"].astype(np.float32)
    outs *= 1.0 / WSCALE  # v carries the host-side W prescale
    outs += bo[None, None, :]
    if _trace or _dump:
        kernel._last_result = res
    return outs

